# revision 14
# baseline (speedup 1.0000x reference)
"""AugmentedLSTMCell on 8 TRN2 NeuronCores — data-parallel over batch.

Layout: feature-on-partition (transposed). Per core: B_loc=2048 batch rows.
  proj.T[j, b] = sum_e W[j, e] * in[b, e]
  Feature-stationary: lhsT = W.T blocks, rhs = in.T [128e, 2048b].
  psum [128j, 2048b] accumulates x-side + h-side contributions
  (the "fused = proj_in + proj_st" add comes free via PSUM accumulation).

Mixed precision for speed (PE fp8 DoubleRow = 2x bf16 ALU throughput):
  fp8 e4m3 (DoubleRow, both operands): i, f, o gates (both sides) and
    hw gate h-side. Weights pre-scaled x64 on host to dodge e4m3
    denormals; every pre-gate activation applies scale=1/64.
  bf16: m gate (both sides), hw gate x-side, hw_proj — these feed the
    output linearly or through tanh (slope 1), so fp8 noise there would
    push rel_err past the 2e-2 gate. bf16 weights also pre-scaled x64
    (exact, power of two) so all psums share the 1/64 activation scale.
  ScalarE applies per-feature bias + sigmoid/tanh straight out of PSUM.
Host transposes outputs back to [B, H].
"""
import sys
import types

sys.path.insert(0, "/opt/trn_rl_repo")
sys.path.insert(0, "/root/.axon_site")

# Shim antenv.axon_hooks (missing on this image) so trace=True can profile.
if "antenv.axon_hooks" not in sys.modules:
    _hooks = types.ModuleType("antenv.axon_hooks")
    _state = {"hook": None}
    _hooks.set_axon_ntff_profile_hook = lambda h: _state.__setitem__("hook", h)
    _hooks.get_axon_ntff_profile_hook = lambda: _state["hook"]
    sys.modules["antenv.axon_hooks"] = _hooks
    try:
        from trn_agent_boot.trn_boot import _ntff_profile_via_ctypes

        _hooks.set_axon_ntff_profile_hook(
            _ntff_profile_via_ctypes("/opt/axon/libaxon_pjrt.so")
        )
    except Exception:
        pass

import numpy as np
import ml_dtypes

import concourse.bass as bass
import concourse.bacc as bacc
import concourse.mybir as mybir
from concourse import tile
from concourse.bass_utils import run_bass_kernel_spmd

BF16 = ml_dtypes.bfloat16
F8NP = ml_dtypes.float8_e4m3

N_CORES = 8
B, E, H = 16384, 1024, 1024
BL = B // N_CORES          # 2048 batch rows per core
KT = E // 128              # 8 contraction k-tiles
NJI = 6 * H // 128         # 48 feature tiles of proj_in
NT = H // 128              # 8 H-slices
BC = 512                   # matmul moving free dim (one PSUM bank)
NBC = BL // BC             # batch chunks per matmul group

WSCALE = 64.0              # weight prescale (host); undone by act scale
INV_WS = 1.0 / WSCALE

# Gate order in Wi rows: i=0, f=1, m=2, o=3, hw=4, hwproj=5.
# x-side fp8 gates [i, f, o, hw] -> wi8 tile index = pos*NT + t
# x-side bf16 gates [m, hwproj] -> wib index
# h-side fp8 gates [i, f, o, hw] -> ws8 index
# h-side bf16 gates [m] -> wsb index
GX8 = [0, 1, 3, 4]
GXB = [2, 5]
GS8 = [0, 1, 3, 4]
GSB = [2]

AF = mybir.ActivationFunctionType
DR = mybir.MatmulPerfMode.DoubleRow


def build_nc():
    nc = bacc.Bacc(None, target_bir_lowering=False)
    f32, bf16, f8 = mybir.dt.float32, mybir.dt.bfloat16, mybir.dt.float8e4

    xbT = nc.declare_dram_parameter("xbT", [E, BL], bf16, isOutput=False)
    x8T = nc.declare_dram_parameter("x8T", [E, BL], f8, isOutput=False)
    h8T = nc.declare_dram_parameter("h8T", [H, BL], f8, isOutput=False)
    hbT = nc.declare_dram_parameter("hbT", [H, BL], bf16, isOutput=False)
    cT = nc.declare_dram_parameter("cT", [H, BL], bf16, isOutput=False)
    wi8 = nc.declare_dram_parameter(
        "wi8", [len(GX8) * NT, 128, KT, 128], f8, isOutput=False
    )
    wib = nc.declare_dram_parameter(
        "wib", [len(GXB) * NT, 128, E], bf16, isOutput=False
    )
    ws8 = nc.declare_dram_parameter(
        "ws8", [len(GS8) * NT, 128, KT, 128], f8, isOutput=False
    )
    wsb = nc.declare_dram_parameter(
        "wsb", [len(GSB) * NT, 128, H], bf16, isOutput=False
    )
    bias = nc.declare_dram_parameter("bias", [128, NJI], f32, isOutput=False)
    outT = nc.declare_dram_parameter("outT", [H, BL], bf16, isOutput=True)
    memT = nc.declare_dram_parameter("memT", [H, BL], bf16, isOutput=True)

    with tile.TileContext(nc) as tc:
        with (
            tc.tile_pool(name="resident", bufs=1) as resident,
            tc.tile_pool(name="wpool8", bufs=6) as wpool8,
            tc.tile_pool(name="wpoolb", bufs=6) as wpoolb,
            tc.tile_pool(name="cpool", bufs=2) as cpool,
            tc.tile_pool(name="psum", bufs=2, space="PSUM") as psum_pool,
            tc.tile_pool(name="gates", bufs=8) as gate_pool,
            tc.tile_pool(name="tmp", bufs=6) as tmp_pool,
            tc.tile_pool(name="outp", bufs=2) as out_pool,
        ):
            # Each dma_start costs ~530ns issue overhead on its engine queue
            # plus ~1.2ns/KB, so: whole-tile DMAs, spread across the queues
            # that are idle at startup (gpsimd/vector), need-order first.
            bias_sb = resident.tile([128, NJI], f32, tag="bias")
            nc.sync.dma_start(bias_sb[:], bias[:])

            # bf16 x resident (hwproj/m/hw x-side). k=0 split for startup.
            xt_k = [None] + [
                resident.tile([128, BL], bf16, tag=f"xt{k}", name=f"xt{k}")
                for k in range(1, KT)
            ]
            xt0a = resident.tile([128, BL // 2], bf16, tag="xt0a", name="xt0a")
            xt0b = resident.tile([128, BL // 2], bf16, tag="xt0b", name="xt0b")

            def rhs_x(k, bc):
                if k == 0:
                    t = xt0a if bc < 2 else xt0b
                    return t[:, (bc % 2) * BC : (bc % 2 + 1) * BC]
                return xt_k[k][:, bc * BC : (bc + 1) * BC]

            ht_k = [
                resident.tile([128, BL], bf16, tag=f"ht{k}", name=f"ht{k}")
                for k in range(KT)
            ]
            # fp8 DoubleRow residents: [128 part, k-tile, batch]
            x8 = resident.tile([128, KT, BL], f8, tag="x8", name="x8")
            h8 = resident.tile([128, KT, BL], f8, tag="h8", name="h8")

            # Preloaded weights for the first feature tiles of t=0.
            w_hwp0 = wpoolb.tile([128, E], bf16, tag="wb")
            w_hwp1 = wpoolb.tile([128, E], bf16, tag="wb")
            w_i0 = wpool8.tile([128, KT, 128], f8, tag="w8")
            w_s0 = wpool8.tile([128, KT, 128], f8, tag="w8")

            # The DMA rings move ~20GB/s each; one dma_start lands on one
            # ring.  Split the startup-critical tiles (first matmuls' deps)
            # into pieces so they ride several rings in parallel; leave the
            # rest as whole-tile DMAs (each costs ~530ns queue issue time).
            def split_dma(dst, src, nsplit, eng):
                n = dst.shape[-1]
                per = n // nsplit
                for q in range(nsplit):
                    sl = slice(q * per, (q + 1) * per)
                    eng.dma_start(dst[..., sl], src[..., sl])

            # gpsimd: bf16 x residents (first tile hwp0, then m0/hw0 x-side)
            split_dma(xt0a, xbT[0:128, : BL // 2], 4, nc.gpsimd)
            split_dma(w_hwp0, wib[1 * NT + 0], 4, nc.sync)
            split_dma(xt0b, xbT[0:128, BL // 2 :], 2, nc.gpsimd)
            for k in range(1, KT):
                nc.gpsimd.dma_start(xt_k[k][:], xbT[k * 128 : (k + 1) * 128, :])
            nc.gpsimd.dma_start(w_hwp1[:], wib[1 * NT + 1])
            # sync: fp8 residents in need-order (i0 x-side first)
            split_dma(w_i0, wi8[0], 2, nc.sync)
            for k in range(KT):
                nsp = 2 if k < 4 else 1
                split_dma(x8[:, k, :], x8T[k * 128 : (k + 1) * 128, :], nsp, nc.sync)
            split_dma(w_s0, ws8[0], 2, nc.sync)
            for k in range(KT):
                nsp = 2 if k < 4 else 1
                split_dma(h8[:, k, :], h8T[k * 128 : (k + 1) * 128, :], nsp, nc.sync)
            # scalar: bf16 h residents (needed from m0's h-side, ~30us in;
            # delays the first activation by ~9us, absorbed by psum bufs=2)
            for k in range(KT):
                nc.scalar.dma_start(ht_k[k][:], hbT[k * 128 : (k + 1) * 128, :])

            def activate(ps, jt, func, chunk_act, width):
                g = gate_pool.tile([128, width], bf16, tag="g")
                cw = width // chunk_act
                for a in range(chunk_act):
                    sl = slice(a * cw, (a + 1) * cw)
                    nc.scalar.activation(
                        g[:, sl], ps[:, sl], func,
                        bias=bias_sb[:, jt : jt + 1], scale=INV_WS,
                    )
                return g

            def ft_fp8(jt, i8, s8, func, w_i=None, w_s=None, chunk_act=1,
                       bc0=0, bc1=NBC):
                """Both sides fp8 DoubleRow."""
                if w_i is None:
                    w_i = wpool8.tile([128, KT, 128], f8, tag="w8")
                    nc.sync.dma_start(w_i[:], wi8[i8])
                if w_s is None:
                    w_s = wpool8.tile([128, KT, 128], f8, tag="w8")
                    nc.sync.dma_start(w_s[:], ws8[s8])
                width = (bc1 - bc0) * BC
                ps = psum_pool.tile([128, width], f32, tag="ps")
                for g in range(KT // 2):
                    for bc in range(bc0, bc1):
                        lo = (bc - bc0) * BC
                        nc.tensor.matmul(
                            ps[:, lo : lo + BC],
                            w_i[:, 2 * g : 2 * g + 2, :],
                            x8[:, 2 * g : 2 * g + 2, bc * BC : (bc + 1) * BC],
                            start=(g == 0), stop=False, perf_mode=DR,
                        )
                for g in range(KT // 2):
                    for bc in range(bc0, bc1):
                        lo = (bc - bc0) * BC
                        nc.tensor.matmul(
                            ps[:, lo : lo + BC],
                            w_s[:, 2 * g : 2 * g + 2, :],
                            h8[:, 2 * g : 2 * g + 2, bc * BC : (bc + 1) * BC],
                            start=False, stop=(g == KT // 2 - 1), perf_mode=DR,
                        )
                return activate(ps, jt, func, chunk_act, width)

            def ft_bf16(jt, ib, sb_i, func, w_i=None, chunk_act=1):
                """Both sides bf16 (sb_i None -> x-side only, e.g. hwproj)."""
                if w_i is None:
                    w_i = wpoolb.tile([128, E], bf16, tag="wb")
                    nc.sync.dma_start(w_i[:], wib[ib])
                has_st = sb_i is not None
                if has_st:
                    w_s = wpoolb.tile([128, H], bf16, tag="wb")
                    nc.sync.dma_start(w_s[:], wsb[sb_i])
                ps = psum_pool.tile([128, BL], f32, tag="ps")
                for k in range(KT):
                    lhsT = w_i[:, k * 128 : (k + 1) * 128]
                    for bc in range(NBC):
                        nc.tensor.matmul(
                            ps[:, bc * BC : (bc + 1) * BC], lhsT, rhs_x(k, bc),
                            start=(k == 0),
                            stop=(not has_st and k == KT - 1),
                        )
                if has_st:
                    for k in range(KT):
                        lhsT = w_s[:, k * 128 : (k + 1) * 128]
                        for bc in range(NBC):
                            nc.tensor.matmul(
                                ps[:, bc * BC : (bc + 1) * BC], lhsT,
                                ht_k[k][:, bc * BC : (bc + 1) * BC],
                                start=False, stop=(k == KT - 1),
                            )
                return activate(ps, jt, func, chunk_act, BL)

            def ft_mixed(jt, ib, s8, func, chunk_act=1, bc0=0, bc1=NBC):
                """x-side bf16, h-side fp8 DoubleRow (hw gate)."""
                w_i = wpoolb.tile([128, E], bf16, tag="wb")
                nc.sync.dma_start(w_i[:], wib[ib])
                w_s = wpool8.tile([128, KT, 128], f8, tag="w8")
                nc.sync.dma_start(w_s[:], ws8[s8])
                width = (bc1 - bc0) * BC
                ps = psum_pool.tile([128, width], f32, tag="ps")
                for k in range(KT):
                    lhsT = w_i[:, k * 128 : (k + 1) * 128]
                    for bc in range(bc0, bc1):
                        lo = (bc - bc0) * BC
                        nc.tensor.matmul(
                            ps[:, lo : lo + BC], lhsT, rhs_x(k, bc),
                            start=(k == 0), stop=False,
                        )
                for g in range(KT // 2):
                    for bc in range(bc0, bc1):
                        lo = (bc - bc0) * BC
                        nc.tensor.matmul(
                            ps[:, lo : lo + BC],
                            w_s[:, 2 * g : 2 * g + 2, :],
                            h8[:, 2 * g : 2 * g + 2, bc * BC : (bc + 1) * BC],
                            start=False, stop=(g == KT // 2 - 1), perf_mode=DR,
                        )
                return activate(ps, jt, func, chunk_act, width)

            mult, addop, subop = (
                mybir.AluOpType.mult,
                mybir.AluOpType.add,
                mybir.AluOpType.subtract,
            )

            hwp_pre = [
                ft_bf16(5 * NT + 0, 1 * NT + 0, None, AF.Identity, w_i=w_hwp0),
                ft_bf16(5 * NT + 1, 1 * NT + 1, None, AF.Identity, w_i=w_hwp1),
            ]
            for t in range(NT):
                hwp = (
                    hwp_pre[t]
                    if t < len(hwp_pre)
                    else ft_bf16(5 * NT + t, 1 * NT + t, None, AF.Identity)
                )
                i_g = ft_fp8(
                    t, 0 * NT + t, 0 * NT + t, AF.Sigmoid,
                    w_i=w_i0 if t == 0 else None,
                    w_s=w_s0 if t == 0 else None,
                )
                f_g = ft_fp8(NT + t, 1 * NT + t, 1 * NT + t, AF.Sigmoid)
                o_g = ft_fp8(3 * NT + t, 2 * NT + t, 2 * NT + t, AF.Sigmoid)
                # m last among the gate tiles: it needs the bf16 residents,
                # which stream in behind the fp8 ones at startup.
                m_g = ft_bf16(2 * NT + t, 0 * NT + t, 0 * NT + t, AF.Tanh)

                ct = cpool.tile([128, BL], bf16, tag="c")
                nc.sync.dma_start(ct[:], cT[t * 128 : (t + 1) * 128, :])

                t1 = tmp_pool.tile([128, BL], bf16, tag="tmp")
                nc.vector.tensor_tensor(t1[:], i_g[:], m_g[:], mult)
                t2 = tmp_pool.tile([128, BL], bf16, tag="tmp")
                nc.vector.tensor_tensor(t2[:], f_g[:], ct[:], mult)
                mem = out_pool.tile([128, BL], bf16, tag="mem")
                nc.vector.tensor_tensor(mem[:], t1[:], t2[:], addop)
                if t < NT - 1:
                    nc.sync.dma_start(memT[t * 128 : (t + 1) * 128, :], mem[:])
                else:
                    # final tile: stripe across rings so the drain is short
                    split_dma(memT[t * 128 : (t + 1) * 128, :], mem, 4, nc.sync)

                tmem = tmp_pool.tile([128, BL], bf16, tag="tmp")
                nc.scalar.activation(tmem[:], mem[:], AF.Tanh)
                outp = tmp_pool.tile([128, BL], bf16, tag="tmp")
                nc.vector.tensor_tensor(outp[:], o_g[:], tmem[:], mult)
                # out = hwp + hw*(outp - hwp), chunked so the tail after the
                # final hw matmuls pipelines with the output DMA.
                u = tmp_pool.tile([128, BL], bf16, tag="tmp")
                nc.vector.tensor_tensor(u[:], outp[:], hwp[:], subop)

                def blend(hw_tile, col0, ncols, nchunk):
                    # out[:, col0:col0+ncols] = hwp + hw*u over `nchunk` pieces
                    ec = ncols // nchunk
                    for e in range(nchunk):
                        sl = slice(col0 + e * ec, col0 + (e + 1) * ec)
                        lsl = slice(e * ec, (e + 1) * ec)
                        v = tmp_pool.tile([128, ec], bf16, tag="v")
                        nc.vector.tensor_tensor(v[:], hw_tile[:, lsl], u[:, sl], mult)
                        outf = out_pool.tile([128, ec], bf16, tag="out")
                        nc.vector.tensor_tensor(outf[:], v[:], hwp[:, sl], addop)
                        nc.sync.dma_start(outT[t * 128 : (t + 1) * 128, sl], outf[:])

                if t < NT - 1:
                    hw_g = ft_fp8(
                        4 * NT + t, 3 * NT + t, 3 * NT + t, AF.Sigmoid,
                        chunk_act=4,
                    )
                    blend(hw_g, 0, BL, 4)
                else:
                    # Last group: split the hw tile in half so the first
                    # half's blend+DMA overlaps the second half's matmuls.
                    for half in range(2):
                        hw_h = ft_fp8(
                            4 * NT + t, 3 * NT + t, 3 * NT + t, AF.Sigmoid,
                            chunk_act=2, bc0=2 * half, bc1=2 * half + 2,
                        )
                        blend(hw_h, half * (BL // 2), BL // 2, 4)

    nc.compile()
    return nc


_NC_CACHE = None


def _get_nc():
    global _NC_CACHE
    if _NC_CACHE is None:
        _NC_CACHE = build_nc()
    return _NC_CACHE


def _pack_trans(W, njt):
    # W [njt*128 j, K e] -> [njt, 128 p, K] with [jt, p, k*128+m] = W[jt*128+m, k*128+p]
    K = W.shape[1]
    kt = K // 128
    return np.ascontiguousarray(
        W.reshape(njt, 128, kt, 128).transpose(0, 3, 2, 1).reshape(njt, 128, K)
    )


def _pack_w_bf16(W, gates):
    # scaled x64, bf16, [n, 128, K]
    blocks = [W[g * H : (g + 1) * H] for g in gates]
    Wb = np.concatenate(blocks, axis=0).astype(np.float32) * WSCALE
    return _pack_trans(Wb, len(gates) * NT).astype(BF16)


def _pack_w_fp8(W, gates):
    # scaled x64, e4m3, [n, 128, KT, 128]
    blocks = [W[g * H : (g + 1) * H] for g in gates]
    Wb = np.concatenate(blocks, axis=0).astype(np.float32) * WSCALE
    n = len(gates) * NT
    K = Wb.shape[1]
    p = _pack_trans(np.clip(Wb, -240, 240), n).astype(F8NP)
    return np.ascontiguousarray(p.reshape(n, 128, K // 128, 128))


def prepare_in_maps(x, h, c, Wi, bi, Ws, bs):
    wi8_p = _pack_w_fp8(np.asarray(Wi, np.float32), GX8)
    wib_p = _pack_w_bf16(np.asarray(Wi, np.float32), GXB)
    ws8_p = _pack_w_fp8(np.asarray(Ws, np.float32), GS8)
    wsb_p = _pack_w_bf16(np.asarray(Ws, np.float32), GSB)
    bias_comb = np.concatenate(
        [np.asarray(bi[: 5 * H], np.float32) + np.asarray(bs, np.float32),
         np.asarray(bi[5 * H :], np.float32)]
    )
    bias_pack = np.ascontiguousarray(bias_comb.reshape(NJI, 128).T).astype(np.float32)

    in_maps = []
    for i in range(N_CORES):
        s = slice(i * BL, (i + 1) * BL)
        xT = np.ascontiguousarray(np.asarray(x[s], np.float32).T)
        hT = np.ascontiguousarray(np.asarray(h[s], np.float32).T)
        in_maps.append(
            {
                "xbT": xT.astype(BF16),
                "x8T": xT.astype(F8NP),
                "h8T": hT.astype(F8NP),
                "hbT": hT.astype(BF16),
                "cT": np.ascontiguousarray(np.asarray(c[s], np.float32).T).astype(BF16),
                "wi8": wi8_p,
                "wib": wib_p,
                "ws8": ws8_p,
                "wsb": wsb_p,
                "bias": bias_pack,
            }
        )
    return in_maps


def run(in_maps, trace=False):
    nc = _get_nc()
    res = run_bass_kernel_spmd(nc, in_maps, core_ids=list(range(N_CORES)), trace=trace)
    out = np.empty((B, H), np.float32)
    mem = np.empty((B, H), np.float32)
    for i in range(N_CORES):
        s = slice(i * BL, (i + 1) * BL)
        out[s] = res.results[i]["outT"].T.astype(np.float32)
        mem[s] = res.results[i]["memT"].T.astype(np.float32)
    return (out, mem), res


def kernel(x, h, c, Wi, bi, Ws, bs):
    in_maps = prepare_in_maps(x, h, c, Wi, bi, Ws, bs)
    (out, mem), _ = run(in_maps, trace=False)
    return out, mem


# revision 15
# speedup vs baseline: 1.0038x; 1.0038x over previous
"""AugmentedLSTMCell on 8 TRN2 NeuronCores — data-parallel over batch.

Layout: feature-on-partition (transposed). Per core: B_loc=2048 batch rows.
  proj.T[j, b] = sum_e W[j, e] * in[b, e]
  Feature-stationary: lhsT = W.T blocks, rhs = in.T [128e, 2048b].
  psum [128j, 2048b] accumulates x-side + h-side contributions
  (the "fused = proj_in + proj_st" add comes free via PSUM accumulation).

Mixed precision for speed (PE fp8 DoubleRow = 2x bf16 ALU throughput):
  fp8 e4m3 (DoubleRow, both operands): i, f, o gates (both sides) and
    hw gate h-side. Weights pre-scaled x64 on host to dodge e4m3
    denormals; every pre-gate activation applies scale=1/64.
  bf16: m gate (both sides), hw gate x-side, hw_proj — these feed the
    output linearly or through tanh (slope 1), so fp8 noise there would
    push rel_err past the 2e-2 gate. bf16 weights also pre-scaled x64
    (exact, power of two) so all psums share the 1/64 activation scale.
  ScalarE applies per-feature bias + sigmoid/tanh straight out of PSUM.
Host transposes outputs back to [B, H].
"""
import sys
import types

sys.path.insert(0, "/opt/trn_rl_repo")
sys.path.insert(0, "/root/.axon_site")

# Shim antenv.axon_hooks (missing on this image) so trace=True can profile.
if "antenv.axon_hooks" not in sys.modules:
    _hooks = types.ModuleType("antenv.axon_hooks")
    _state = {"hook": None}
    _hooks.set_axon_ntff_profile_hook = lambda h: _state.__setitem__("hook", h)
    _hooks.get_axon_ntff_profile_hook = lambda: _state["hook"]
    sys.modules["antenv.axon_hooks"] = _hooks
    try:
        from trn_agent_boot.trn_boot import _ntff_profile_via_ctypes

        _hooks.set_axon_ntff_profile_hook(
            _ntff_profile_via_ctypes("/opt/axon/libaxon_pjrt.so")
        )
    except Exception:
        pass

import numpy as np
import ml_dtypes

import concourse.bass as bass
import concourse.bacc as bacc
import concourse.mybir as mybir
from concourse import tile
from concourse.bass_utils import run_bass_kernel_spmd

BF16 = ml_dtypes.bfloat16
F8NP = ml_dtypes.float8_e4m3

N_CORES = 8
B, E, H = 16384, 1024, 1024
BL = B // N_CORES          # 2048 batch rows per core
KT = E // 128              # 8 contraction k-tiles
NJI = 6 * H // 128         # 48 feature tiles of proj_in
NT = H // 128              # 8 H-slices
BC = 512                   # matmul moving free dim (one PSUM bank)
NBC = BL // BC             # batch chunks per matmul group

WSCALE = 64.0              # weight prescale (host); undone by act scale
INV_WS = 1.0 / WSCALE

# Gate order in Wi rows: i=0, f=1, m=2, o=3, hw=4, hwproj=5.
# x-side fp8 gates [i, f, o, hw] -> wi8 tile index = pos*NT + t
# x-side bf16 gates [m, hwproj] -> wib index
# h-side fp8 gates [i, f, o, hw] -> ws8 index
# h-side bf16 gates [m] -> wsb index
GX8 = [0, 1, 3, 4]
GXB = [2, 5]
GS8 = [0, 1, 3, 4]
GSB = [2]

AF = mybir.ActivationFunctionType
DR = mybir.MatmulPerfMode.DoubleRow


def build_nc():
    nc = bacc.Bacc(None, target_bir_lowering=False)
    f32, bf16, f8 = mybir.dt.float32, mybir.dt.bfloat16, mybir.dt.float8e4

    xbT = nc.declare_dram_parameter("xbT", [E, BL], bf16, isOutput=False)
    x8T = nc.declare_dram_parameter("x8T", [E, BL], f8, isOutput=False)
    h8T = nc.declare_dram_parameter("h8T", [H, BL], f8, isOutput=False)
    hbT = nc.declare_dram_parameter("hbT", [H, BL], bf16, isOutput=False)
    cT = nc.declare_dram_parameter("cT", [H, BL], bf16, isOutput=False)
    wi8 = nc.declare_dram_parameter(
        "wi8", [len(GX8) * NT, 128, KT, 128], f8, isOutput=False
    )
    wib = nc.declare_dram_parameter(
        "wib", [len(GXB) * NT, 128, E], bf16, isOutput=False
    )
    ws8 = nc.declare_dram_parameter(
        "ws8", [len(GS8) * NT, 128, KT, 128], f8, isOutput=False
    )
    wsb = nc.declare_dram_parameter(
        "wsb", [len(GSB) * NT, 128, H], bf16, isOutput=False
    )
    bias = nc.declare_dram_parameter("bias", [128, NJI], f32, isOutput=False)
    outT = nc.declare_dram_parameter("outT", [H, BL], bf16, isOutput=True)
    memT = nc.declare_dram_parameter("memT", [H, BL], bf16, isOutput=True)

    with tile.TileContext(nc) as tc:
        with (
            tc.tile_pool(name="resident", bufs=1) as resident,
            tc.tile_pool(name="wpool8", bufs=6) as wpool8,
            tc.tile_pool(name="wpoolb", bufs=6) as wpoolb,
            tc.tile_pool(name="cpool", bufs=2) as cpool,
            tc.tile_pool(name="psum", bufs=2, space="PSUM") as psum_pool,
            tc.tile_pool(name="gates", bufs=8) as gate_pool,
            tc.tile_pool(name="tmp", bufs=6) as tmp_pool,
            tc.tile_pool(name="outp", bufs=2) as out_pool,
        ):
            # Each dma_start costs ~530ns issue overhead on its engine queue
            # plus ~1.2ns/KB, so: whole-tile DMAs, spread across the queues
            # that are idle at startup (gpsimd/vector), need-order first.
            bias_sb = resident.tile([128, NJI], f32, tag="bias")
            nc.sync.dma_start(bias_sb[:], bias[:])

            # bf16 x resident (hwproj/m/hw x-side). k=0 split for startup.
            xt_k = [None] + [
                resident.tile([128, BL], bf16, tag=f"xt{k}", name=f"xt{k}")
                for k in range(1, KT)
            ]
            xt0a = resident.tile([128, BL // 2], bf16, tag="xt0a", name="xt0a")
            xt0b = resident.tile([128, BL // 2], bf16, tag="xt0b", name="xt0b")

            def rhs_x(k, bc):
                if k == 0:
                    t = xt0a if bc < 2 else xt0b
                    return t[:, (bc % 2) * BC : (bc % 2 + 1) * BC]
                return xt_k[k][:, bc * BC : (bc + 1) * BC]

            ht_k = [
                resident.tile([128, BL], bf16, tag=f"ht{k}", name=f"ht{k}")
                for k in range(KT)
            ]
            # fp8 DoubleRow residents: [128 part, k-tile, batch]
            x8 = resident.tile([128, KT, BL], f8, tag="x8", name="x8")
            h8 = resident.tile([128, KT, BL], f8, tag="h8", name="h8")

            # Preloaded weights for the first feature tiles of t=0.
            w_hwp0 = wpoolb.tile([128, E], bf16, tag="wb")
            w_hwp1 = wpoolb.tile([128, E], bf16, tag="wb")
            w_i0 = wpool8.tile([128, KT, 128], f8, tag="w8")
            w_s0 = wpool8.tile([128, KT, 128], f8, tag="w8")

            # The DMA rings move ~20GB/s each; one dma_start lands on one
            # ring.  Split the startup-critical tiles (first matmuls' deps)
            # into pieces so they ride several rings in parallel; leave the
            # rest as whole-tile DMAs (each costs ~530ns queue issue time).
            def split_dma(dst, src, nsplit, eng):
                n = dst.shape[-1]
                per = n // nsplit
                for q in range(nsplit):
                    sl = slice(q * per, (q + 1) * per)
                    eng.dma_start(dst[..., sl], src[..., sl])

            # gpsimd: bf16 x residents (first tile hwp0, then m0/hw0 x-side)
            split_dma(xt0a, xbT[0:128, : BL // 2], 4, nc.gpsimd)
            split_dma(w_hwp0, wib[1 * NT + 0], 4, nc.sync)
            split_dma(xt0b, xbT[0:128, BL // 2 :], 2, nc.gpsimd)
            for k in range(1, KT):
                nc.gpsimd.dma_start(xt_k[k][:], xbT[k * 128 : (k + 1) * 128, :])
            nc.gpsimd.dma_start(w_hwp1[:], wib[1 * NT + 1])
            # sync: fp8 residents in need-order (i0 x-side first)
            split_dma(w_i0, wi8[0], 2, nc.sync)
            for k in range(KT):
                nsp = 2 if k < 4 else 1
                split_dma(x8[:, k, :], x8T[k * 128 : (k + 1) * 128, :], nsp, nc.sync)
            split_dma(w_s0, ws8[0], 2, nc.sync)
            for k in range(KT):
                nsp = 2 if k < 4 else 1
                split_dma(h8[:, k, :], h8T[k * 128 : (k + 1) * 128, :], nsp, nc.sync)
            # scalar: bf16 h residents (needed from m0's h-side, ~30us in;
            # delays the first activation by ~9us, absorbed by psum bufs=2)
            for k in range(KT):
                nc.scalar.dma_start(ht_k[k][:], hbT[k * 128 : (k + 1) * 128, :])

            def activate(ps, jt, func, chunk_act, width):
                g = gate_pool.tile([128, width], bf16, tag="g")
                cw = width // chunk_act
                for a in range(chunk_act):
                    sl = slice(a * cw, (a + 1) * cw)
                    nc.scalar.activation(
                        g[:, sl], ps[:, sl], func,
                        bias=bias_sb[:, jt : jt + 1], scale=INV_WS,
                    )
                return g

            def ft_fp8(jt, i8, s8, func, w_i=None, w_s=None, chunk_act=1,
                       bc0=0, bc1=NBC):
                """Both sides fp8 DoubleRow."""
                if w_i is None:
                    w_i = wpool8.tile([128, KT, 128], f8, tag="w8")
                    nc.sync.dma_start(w_i[:], wi8[i8])
                if w_s is None:
                    w_s = wpool8.tile([128, KT, 128], f8, tag="w8")
                    nc.sync.dma_start(w_s[:], ws8[s8])
                width = (bc1 - bc0) * BC
                ps = psum_pool.tile([128, width], f32, tag="ps")
                for g in range(KT // 2):
                    for bc in range(bc0, bc1):
                        lo = (bc - bc0) * BC
                        nc.tensor.matmul(
                            ps[:, lo : lo + BC],
                            w_i[:, 2 * g : 2 * g + 2, :],
                            x8[:, 2 * g : 2 * g + 2, bc * BC : (bc + 1) * BC],
                            start=(g == 0), stop=False, perf_mode=DR,
                        )
                for g in range(KT // 2):
                    for bc in range(bc0, bc1):
                        lo = (bc - bc0) * BC
                        nc.tensor.matmul(
                            ps[:, lo : lo + BC],
                            w_s[:, 2 * g : 2 * g + 2, :],
                            h8[:, 2 * g : 2 * g + 2, bc * BC : (bc + 1) * BC],
                            start=False, stop=(g == KT // 2 - 1), perf_mode=DR,
                        )
                return activate(ps, jt, func, chunk_act, width)

            def ft_bf16(jt, ib, sb_i, func, w_i=None, chunk_act=1):
                """Both sides bf16 (sb_i None -> x-side only, e.g. hwproj)."""
                if w_i is None:
                    w_i = wpoolb.tile([128, E], bf16, tag="wb")
                    nc.sync.dma_start(w_i[:], wib[ib])
                has_st = sb_i is not None
                if has_st:
                    w_s = wpoolb.tile([128, H], bf16, tag="wb")
                    nc.sync.dma_start(w_s[:], wsb[sb_i])
                ps = psum_pool.tile([128, BL], f32, tag="ps")
                for k in range(KT):
                    lhsT = w_i[:, k * 128 : (k + 1) * 128]
                    for bc in range(NBC):
                        nc.tensor.matmul(
                            ps[:, bc * BC : (bc + 1) * BC], lhsT, rhs_x(k, bc),
                            start=(k == 0),
                            stop=(not has_st and k == KT - 1),
                        )
                if has_st:
                    for k in range(KT):
                        lhsT = w_s[:, k * 128 : (k + 1) * 128]
                        for bc in range(NBC):
                            nc.tensor.matmul(
                                ps[:, bc * BC : (bc + 1) * BC], lhsT,
                                ht_k[k][:, bc * BC : (bc + 1) * BC],
                                start=False, stop=(k == KT - 1),
                            )
                return activate(ps, jt, func, chunk_act, BL)

            def ft_mixed(jt, ib, s8, func, chunk_act=1, bc0=0, bc1=NBC):
                """x-side bf16, h-side fp8 DoubleRow (hw gate)."""
                w_i = wpoolb.tile([128, E], bf16, tag="wb")
                nc.sync.dma_start(w_i[:], wib[ib])
                w_s = wpool8.tile([128, KT, 128], f8, tag="w8")
                nc.sync.dma_start(w_s[:], ws8[s8])
                width = (bc1 - bc0) * BC
                ps = psum_pool.tile([128, width], f32, tag="ps")
                for k in range(KT):
                    lhsT = w_i[:, k * 128 : (k + 1) * 128]
                    for bc in range(bc0, bc1):
                        lo = (bc - bc0) * BC
                        nc.tensor.matmul(
                            ps[:, lo : lo + BC], lhsT, rhs_x(k, bc),
                            start=(k == 0), stop=False,
                        )
                for g in range(KT // 2):
                    for bc in range(bc0, bc1):
                        lo = (bc - bc0) * BC
                        nc.tensor.matmul(
                            ps[:, lo : lo + BC],
                            w_s[:, 2 * g : 2 * g + 2, :],
                            h8[:, 2 * g : 2 * g + 2, bc * BC : (bc + 1) * BC],
                            start=False, stop=(g == KT // 2 - 1), perf_mode=DR,
                        )
                return activate(ps, jt, func, chunk_act, width)

            mult, addop, subop = (
                mybir.AluOpType.mult,
                mybir.AluOpType.add,
                mybir.AluOpType.subtract,
            )

            hwp_pre = [
                ft_bf16(5 * NT + 0, 1 * NT + 0, None, AF.Identity, w_i=w_hwp0),
                ft_bf16(5 * NT + 1, 1 * NT + 1, None, AF.Identity, w_i=w_hwp1),
            ]
            for t in range(NT):
                hwp = (
                    hwp_pre[t]
                    if t < len(hwp_pre)
                    else ft_bf16(5 * NT + t, 1 * NT + t, None, AF.Identity)
                )
                i_g = ft_fp8(
                    t, 0 * NT + t, 0 * NT + t, AF.Sigmoid,
                    w_i=w_i0 if t == 0 else None,
                    w_s=w_s0 if t == 0 else None,
                )
                f_g = ft_fp8(NT + t, 1 * NT + t, 1 * NT + t, AF.Sigmoid)
                o_g = ft_fp8(3 * NT + t, 2 * NT + t, 2 * NT + t, AF.Sigmoid)
                # m last among the gate tiles: it needs the bf16 residents,
                # which stream in behind the fp8 ones at startup.
                m_g = ft_bf16(2 * NT + t, 0 * NT + t, 0 * NT + t, AF.Tanh)

                ct = cpool.tile([128, BL], bf16, tag="c")
                nc.sync.dma_start(ct[:], cT[t * 128 : (t + 1) * 128, :])

                t1 = tmp_pool.tile([128, BL], bf16, tag="tmp")
                nc.vector.tensor_tensor(t1[:], i_g[:], m_g[:], mult)
                t2 = tmp_pool.tile([128, BL], bf16, tag="tmp")
                nc.vector.tensor_tensor(t2[:], f_g[:], ct[:], mult)
                mem = out_pool.tile([128, BL], bf16, tag="mem")
                nc.vector.tensor_tensor(mem[:], t1[:], t2[:], addop)
                if t < NT - 1:
                    nc.sync.dma_start(memT[t * 128 : (t + 1) * 128, :], mem[:])
                else:
                    # final tile: stripe across rings so the drain is short
                    split_dma(memT[t * 128 : (t + 1) * 128, :], mem, 4, nc.sync)

                tmem = tmp_pool.tile([128, BL], bf16, tag="tmp")
                nc.scalar.activation(tmem[:], mem[:], AF.Tanh)
                outp = tmp_pool.tile([128, BL], bf16, tag="tmp")
                nc.vector.tensor_tensor(outp[:], o_g[:], tmem[:], mult)
                # out = hwp + hw*(outp - hwp), chunked so the tail after the
                # final hw matmuls pipelines with the output DMA.
                u = tmp_pool.tile([128, BL], bf16, tag="tmp")
                nc.vector.tensor_tensor(u[:], outp[:], hwp[:], subop)

                def blend(hw_tile, col0, ncols, nchunk):
                    # out[:, col0:col0+ncols] = hwp + hw*u over `nchunk` pieces
                    ec = ncols // nchunk
                    for e in range(nchunk):
                        sl = slice(col0 + e * ec, col0 + (e + 1) * ec)
                        lsl = slice(e * ec, (e + 1) * ec)
                        v = tmp_pool.tile([128, ec], bf16, tag="v")
                        nc.vector.tensor_tensor(v[:], hw_tile[:, lsl], u[:, sl], mult)
                        outf = out_pool.tile([128, ec], bf16, tag="out")
                        nc.vector.tensor_tensor(outf[:], v[:], hwp[:, sl], addop)
                        nc.sync.dma_start(outT[t * 128 : (t + 1) * 128, sl], outf[:])

                if t < NT - 1:
                    hw_g = ft_fp8(
                        4 * NT + t, 3 * NT + t, 3 * NT + t, AF.Sigmoid,
                        chunk_act=4,
                    )
                    blend(hw_g, 0, BL, 4)
                else:
                    # Last group: split the hw tile into quarters so each
                    # quarter's act+blend+DMA overlaps the next quarter's
                    # matmuls, shrinking the post-last-matmul tail.
                    w_hw_i = wpool8.tile([128, KT, 128], f8, tag="w8")
                    nc.sync.dma_start(w_hw_i[:], wi8[3 * NT + t])
                    w_hw_s = wpool8.tile([128, KT, 128], f8, tag="w8")
                    nc.sync.dma_start(w_hw_s[:], ws8[3 * NT + t])
                    for q in range(NBC):
                        hw_q = ft_fp8(
                            4 * NT + t, 3 * NT + t, 3 * NT + t, AF.Sigmoid,
                            w_i=w_hw_i, w_s=w_hw_s,
                            chunk_act=1, bc0=q, bc1=q + 1,
                        )
                        blend(hw_q, q * BC, BC, 2)

    nc.compile()
    return nc


_NC_CACHE = None


def _get_nc():
    global _NC_CACHE
    if _NC_CACHE is None:
        _NC_CACHE = build_nc()
    return _NC_CACHE


def _pack_trans(W, njt):
    # W [njt*128 j, K e] -> [njt, 128 p, K] with [jt, p, k*128+m] = W[jt*128+m, k*128+p]
    K = W.shape[1]
    kt = K // 128
    return np.ascontiguousarray(
        W.reshape(njt, 128, kt, 128).transpose(0, 3, 2, 1).reshape(njt, 128, K)
    )


def _pack_w_bf16(W, gates):
    # scaled x64, bf16, [n, 128, K]
    blocks = [W[g * H : (g + 1) * H] for g in gates]
    Wb = np.concatenate(blocks, axis=0).astype(np.float32) * WSCALE
    return _pack_trans(Wb, len(gates) * NT).astype(BF16)


def _pack_w_fp8(W, gates):
    # scaled x64, e4m3, [n, 128, KT, 128]
    blocks = [W[g * H : (g + 1) * H] for g in gates]
    Wb = np.concatenate(blocks, axis=0).astype(np.float32) * WSCALE
    n = len(gates) * NT
    K = Wb.shape[1]
    p = _pack_trans(np.clip(Wb, -240, 240), n).astype(F8NP)
    return np.ascontiguousarray(p.reshape(n, 128, K // 128, 128))


def prepare_in_maps(x, h, c, Wi, bi, Ws, bs):
    wi8_p = _pack_w_fp8(np.asarray(Wi, np.float32), GX8)
    wib_p = _pack_w_bf16(np.asarray(Wi, np.float32), GXB)
    ws8_p = _pack_w_fp8(np.asarray(Ws, np.float32), GS8)
    wsb_p = _pack_w_bf16(np.asarray(Ws, np.float32), GSB)
    bias_comb = np.concatenate(
        [np.asarray(bi[: 5 * H], np.float32) + np.asarray(bs, np.float32),
         np.asarray(bi[5 * H :], np.float32)]
    )
    bias_pack = np.ascontiguousarray(bias_comb.reshape(NJI, 128).T).astype(np.float32)

    in_maps = []
    for i in range(N_CORES):
        s = slice(i * BL, (i + 1) * BL)
        xT = np.ascontiguousarray(np.asarray(x[s], np.float32).T)
        hT = np.ascontiguousarray(np.asarray(h[s], np.float32).T)
        in_maps.append(
            {
                "xbT": xT.astype(BF16),
                "x8T": xT.astype(F8NP),
                "h8T": hT.astype(F8NP),
                "hbT": hT.astype(BF16),
                "cT": np.ascontiguousarray(np.asarray(c[s], np.float32).T).astype(BF16),
                "wi8": wi8_p,
                "wib": wib_p,
                "ws8": ws8_p,
                "wsb": wsb_p,
                "bias": bias_pack,
            }
        )
    return in_maps


def run(in_maps, trace=False):
    nc = _get_nc()
    res = run_bass_kernel_spmd(nc, in_maps, core_ids=list(range(N_CORES)), trace=trace)
    out = np.empty((B, H), np.float32)
    mem = np.empty((B, H), np.float32)
    for i in range(N_CORES):
        s = slice(i * BL, (i + 1) * BL)
        out[s] = res.results[i]["outT"].T.astype(np.float32)
        mem[s] = res.results[i]["memT"].T.astype(np.float32)
    return (out, mem), res


def kernel(x, h, c, Wi, bi, Ws, bs):
    in_maps = prepare_in_maps(x, h, c, Wi, bi, Ws, bs)
    (out, mem), _ = run(in_maps, trace=False)
    return out, mem
